# revision 13
# baseline (speedup 1.0000x reference)
"""Dilated block attention + output projection on 8 trn2 cores.

Sharding: core c handles batch b = c//2 and heads h = 4*(c%2) .. +3.
Each core computes the full dilated-attention combine for its 4 (b,h)
pairs and a partial output projection (contraction over its 4 heads'
256 hidden dims).  The host sums the two half-hidden partials per batch
and adds the bias.

Math note: the reference's stabilized-softmax + detached-expsum
reweighting collapses to the unstabilized form
    out[p] = (sum_d exp(S_d) @ V_d  scattered to p) / (sum_d rowsum exp(S_d))
which is what the kernel computes (scores ~ N(0,1), no overflow risk).

v2 engine plan (from trace analysis of v1: ScalarE ACT was 82% busy on
exp, PE ran matmuls serially at ~427ns):
  - exp is split between ScalarE (activation Exp) and VectorE, which
    runs a custom-DVE exp: v = int32(s*c1 + c2) packs n=floor(y) into
    the exponent field and frac(y) into the mantissa; one fused DVE op
    unpacks p=2^n (AND -inf), z=1+f (XOR p, OR 1.0) and evaluates the
    monic minimax quadratic E = ((z+A)z+B)*p.  Max rel err 1.7e-3,
    below the bf16 output quantization.
  - QK pairs run parity-split (even k-tiles on partitions 0-63, odd on
    64-127) so consecutive K=64 matmuls overlap: 216ns per pair.
  - PV keeps the ones-column (M=65, K=128; rowsum lands on psum row 64
    for free); K=128 back-to-back matmuls hide LDWEIGHTS (216ns).
  - o_proj contracts TWO heads per matmul: oacc for heads (0,1) and
    (2,3) are stacked into [128, L] tiles, K=128 against Wo^T slices.
  - normalization runs on the otherwise-idle GPSIMD: 1/w via fast DVE
    reciprocal in [32,128] shape, partition_broadcast to [64, L], one
    big GPSIMD tensor_mul per pair.
"""

import ml_dtypes
import numpy as np

BF16_NP = ml_dtypes.bfloat16

B, H, L, HD = 4, 8, 4096, 64
HIDDEN = H * HD
DILS = (1, 2, 4, 8)
BLOCK = 1024
PB = 4  # (b,h) pairs per core
NCORES = 8
LDS = [L // d for d in DILS]  # 4096, 2048, 1024, 512
NKTS = [ld // 128 for ld in LDS]  # 32, 16, 8, 4
# blob widths per branch: Q dup (Ld) + K parity-split (Ld/2) + V slabs
# (nkt*128; V is padded 65 -> 128 cols so PV LDWEIGHTS gets fast-weight-load)
WS = [ld + ld // 2 + nkt * 128 for ld, nkt in zip(LDS, NKTS)]
BOFFS = [sum(WS[:i]) for i in range(len(WS))]
WSUM = sum(WS)
QCH = 512  # q-chunk (strided-domain positions) per psum window

# fast-exp constants: exp(0.125*s) = 2^n * ((z+A)*z + B), see module docstring
EXP_A = -0.04965337575705925
EXP_B = 2.0204604655775418
EXP_C1 = float(np.float32(1512775.375))  # 0.125*log2(e)*2^23
EXP_C2 = float(np.float32(1052196800.0))  # (127 + delta)*2^23

# fraction of exp jobs routed to the DVE path (rest on ScalarE)
DVE_NUM, DVE_DEN = 57, 232

_PROGRAM = None


def register_exp2_op():
    """Register the custom-DVE fused exp2-unpack op (idempotent)."""
    import concourse.dve_ops as dve_ops

    for op in dve_ops.OPS:
        if op.name == "EXP2_BITS_ANT":
            return op
    from concourse.dve_spec import (
        AluOp,
        Bin,
        C0,
        C1,
        C2,
        One,
        Spec,
        Src0,
        lower,
        _has_src1,
    )
    from concourse.dve_uop import DveOpSpec

    name = "EXP2_BITS_ANT"
    p = Bin(AluOp.BITWISE_AND, Src0, C2)  # C2 = -inf: bits 0xFF800000
    z = Bin(AluOp.BITWISE_OR, Bin(AluOp.BITWISE_XOR, Src0, p), One)
    body = ((z + C0) * z + C1) * p

    def ref(in0, in1, c0, c1, c2):
        v = np.asarray(in0)
        vb = v.view(np.int32) if v.dtype == np.float32 else v.astype(np.int32)
        pb = vb & np.int32(np.uint32(0xFF800000))
        zb = (vb ^ pb) | np.float32(1.0).view(np.int32)
        pf = pb.view(np.float32)
        zf = zb.view(np.float32)
        return (((zf + c0) * zf + c1) * pf).astype(np.float32)

    spec = Spec(body=body, reference=ref)
    shas = {}
    for ver in ("v3", "v4"):
        tmp = DveOpSpec(
            name=name, opcode=0, uops=lower(spec, ver=ver), rd1_en=_has_src1(spec)
        )
        shas[ver] = tmp.sha(ver)
    op = dve_ops.DveOp(name, spec, subdim=False, uops_sha=shas)
    dve_ops.OPS.append(op)
    dve_ops._SUB_OPCODE_FOR_NAME[name] = (
        max(dve_ops._SUB_OPCODE_FOR_NAME.values()) + 1
    )
    dve_ops.CUSTOM_DVE_SPECS[name] = spec
    return op


def build_program():
    """Build the (SPMD, identical on all cores) Bass program."""
    from contextlib import ExitStack

    import concourse.tile as tile
    from concourse import bacc, mybir

    F32 = mybir.dt.float32
    I32 = mybir.dt.int32
    BF16 = mybir.dt.bfloat16
    exp_op = register_exp2_op()
    nc = bacc.Bacc("TRN2", target_bir_lowering=False, debug=False)

    blob_d = nc.dram_tensor("blob", [PB, 128, WSUM], BF16, kind="ExternalInput")
    wtmp_d = nc.dram_tensor("wtmp", [PB, 32, 128], F32, kind="Internal")
    wtmp2_d = nc.dram_tensor("wtmp2", [PB, 32, 128], BF16, kind="Internal")
    wot_d = nc.dram_tensor("wot", [2, 128, HIDDEN], BF16, kind="ExternalInput")
    out_d = nc.dram_tensor("out", [L, HIDDEN], BF16, kind="ExternalOutput")

    with tile.TileContext(nc) as tc, ExitStack() as ctx:
        consts = ctx.enter_context(tc.tile_pool(name="consts", bufs=1))
        br_pool = ctx.enter_context(tc.tile_pool(name="br", bufs=1))
        e_pool = ctx.enter_context(tc.tile_pool(name="ep", bufs=8))
        v_pool = ctx.enter_context(tc.tile_pool(name="vp", bufs=2))
        acc_pool = ctx.enter_context(tc.tile_pool(name="accp", bufs=1))
        io_pool = ctx.enter_context(tc.tile_pool(name="iop", bufs=2))
        st_psum = ctx.enter_context(tc.tile_pool(name="stp", bufs=2, space="PSUM"))
        st_psum_dve = ctx.enter_context(
            tc.tile_pool(name="stpd", bufs=1, space="PSUM")
        )
        pv_psum = ctx.enter_context(tc.tile_pool(name="pvp", bufs=2, space="PSUM"))

        zero_bias = consts.tile([128, 1], F32, tag="zb")
        nc.vector.memset(zero_bias, 0.0)
        ones_row = consts.tile([128, 64], F32, tag="ones_row")
        nc.vector.memset(ones_row, 1.0)

        wot_sb = consts.tile([128, 2, HIDDEN], BF16, tag="wot")

        acc_tiles = [
            acc_pool.tile([65, L], F32, tag=f"acc{j}", bufs=1, name=f"acc{j}")
            for j in range(PB)
        ]
        # oacc for heads (0,1) stacked on partitions 0-63 / 64-127, same (2,3)
        oacc_tiles = [
            acc_pool.tile([128, L], BF16, tag=f"oacc{g}", bufs=1, name=f"oacc{g}")
            for g in range(2)
        ]
        bc_tiles = [
            acc_pool.tile([64, L], BF16, tag=f"bc{j % 2}", bufs=1, name=f"bc{j}")
            for j in range(2)
        ]

        job_seq = [0]  # global exp-job counter for the ACT/DVE split
        prefetched = {}

        def prefetch_next_pair(jj):
            """DMA the next pair's first branch blob early (QK + V tiles)."""
            di = len(DILS) - 1 if jj == PB - 1 else 0
            bufs = 1 if di <= 1 else 2
            wqk = LDS[di] + LDS[di] // 2
            bt = br_pool.tile(
                [128, wqk], BF16, tag=f"b{di}", bufs=bufs, name=f"bt{di}"
            )
            nc.sync.dma_start(
                out=bt, in_=blob_d[jj, :, BOFFS[di] : BOFFS[di] + wqk]
            )
            btv = br_pool.tile(
                [128, NKTS[di] * 128], BF16, tag=f"bv{di}", bufs=bufs,
                name=f"btv{di}",
            )
            nc.sync.dma_start(
                out=btv,
                in_=blob_d[jj, :, BOFFS[di] + wqk : BOFFS[di] + WS[di]],
            )
            prefetched[(jj, di)] = (bt, btv)

        for j in range(PB):
            acc = acc_tiles[j]

            jobs = []
            bt_tiles = {}
            di_order = (
                list(range(len(DILS)))
                if j < PB - 1
                else list(range(len(DILS) - 1, -1, -1))
            )
            for di in di_order:
                d = DILS[di]
                Ld = LDS[di]
                bs = min(BLOCK, Ld)
                nblk = Ld // bs
                nkt_blk = bs // 128
                for blk in range(nblk):
                    for qc in range(bs // QCH):
                        q0 = blk * bs + qc * QCH
                        kts = list(range(nkt_blk))
                        groups = [kts[x : x + 2] for x in range(0, nkt_blk, 2)]
                        for gi, g in enumerate(groups):
                            jobs.append(
                                dict(
                                    di=di,
                                    d=d,
                                    blk=blk,
                                    nkt_blk=nkt_blk,
                                    q0=q0,
                                    g=g,
                                    first=(gi == 0),
                                    last=(gi == len(groups) - 1),
                                    done0=sum(len(x) for x in groups[:gi]),
                                )
                            )

            def get_bt(di):
                if di not in bt_tiles and (j, di) in prefetched:
                    bt_tiles[di] = prefetched.pop((j, di))
                if di not in bt_tiles:
                    bufs = 1 if di <= 1 else 2
                    wqk = LDS[di] + LDS[di] // 2
                    bt = br_pool.tile(
                        [128, wqk], BF16, tag=f"b{di}", bufs=bufs, name=f"bt{di}"
                    )
                    nc.sync.dma_start(
                        out=bt, in_=blob_d[j, :, BOFFS[di] : BOFFS[di] + wqk]
                    )
                    btv = br_pool.tile(
                        [128, NKTS[di] * 128], BF16, tag=f"bv{di}", bufs=bufs,
                        name=f"btv{di}",
                    )
                    nc.sync.dma_start(
                        out=btv,
                        in_=blob_d[j, :, BOFFS[di] + wqk : BOFFS[di] + WS[di]],
                    )
                    bt_tiles[di] = (bt, btv)
                return bt_tiles[di]

            get_bt(jobs[0]["di"])
            for jb in jobs[1:]:
                if jb["di"] != jobs[0]["di"]:
                    get_bt(jb["di"])
                    break
            if j == 0:
                nc.sync.dma_start(
                    out=wot_sb, in_=wot_d.rearrange("g r c -> r g c")
                )
            if j == PB - 1:
                nc.gpsimd.memset(acc, 0.0)

            def emit_qk_exp(job):
                """QK matmuls for the group -> exp to a bf16 E tile.

                Exp runs on ScalarE or (for a Bresenham-selected subset)
                on VectorE via int32-convert + custom exp2 op."""
                di, q0, g = job["di"], job["q0"], job["g"]
                Ld = LDS[di]
                kbase = Ld
                bt, _ = get_bt(di)
                n = job_seq[0]
                job_seq[0] += 1
                use_dve = ((n * DVE_NUM) // DVE_DEN) != (
                    ((n + 1) * DVE_NUM) // DVE_DEN
                )
                pool = st_psum_dve if use_dve else st_psum
                st = pool.tile([128, 2, QCH], F32, tag="st", name="st")
                for i, kt in enumerate(g):
                    tg = job["blk"] * job["nkt_blk"] + kt
                    half = tg % 2
                    k0 = kbase + (tg // 2) * 128
                    nc.tensor.matmul(
                        st[:, i, :],
                        bt[half * 64 : (half + 1) * 64, k0 : k0 + 128],
                        bt[half * 64 : (half + 1) * 64, q0 : q0 + QCH],
                        start=True,
                        stop=True,
                    )
                et = e_pool.tile([128, 2, QCH], BF16, tag="et", name="et")
                if use_dve:
                    vt = v_pool.tile([128, 2, QCH], I32, tag="vt", name="vt")
                    nc.vector.tensor_scalar(
                        out=vt,
                        in0=st,
                        scalar1=EXP_C1,
                        scalar2=EXP_C2,
                        op0=mybir.AluOpType.mult,
                        op1=mybir.AluOpType.add,
                    )
                    nc.vector._custom_dve(
                        exp_op,
                        out=et[:, :, :],
                        in0=vt[:, :, :].bitcast(F32),
                        s0=EXP_A,
                        s1=EXP_B,
                        imm2=float("-inf"),
                    )
                else:
                    nc.scalar.activation(
                        et,
                        st,
                        mybir.ActivationFunctionType.Exp,
                        bias=zero_bias,
                        scale=0.125,
                    )
                job["et"] = et

            def emit_pv(job):
                """PV accumulation for the group; combine if window done."""
                di, d = job["di"], job["d"]
                Ld = LDS[di]
                _, btv = get_bt(di)
                et = job["et"]
                pv = job["pv"]
                done = job["done0"]
                for i, kt in enumerate(job["g"]):
                    tg = job["blk"] * job["nkt_blk"] + kt
                    nc.tensor.matmul(
                        pv[0:128, :],
                        btv[:, tg * 128 : tg * 128 + 128],
                        et[:, i, :],
                        start=(done == 0),
                        stop=(done == job["nkt_blk"] - 1),
                        skip_group_check=True,
                    )
                    done += 1
                if job["last"]:
                    p0 = job["q0"] * d
                    if d == 1 and j < PB - 1:
                        nc.vector.tensor_copy(
                            out=acc[:, p0 : p0 + QCH], in_=pv[0:65, :]
                        )
                    else:
                        dst = acc[:, p0 : p0 + QCH * d : d]
                        nc.vector.tensor_add(out=dst, in0=dst, in1=pv[0:65, :])
                    if j == PB - 1 and d == 1:
                        tail_ready.append(job["q0"] // QCH)
                        if len(tail_ready) > 2:
                            emit_tail_window(tail_ready.pop(0))

            def emit_tail_window(w):
                """Normalize pair PB-1 window w and run o_proj for its
                4 M-tiles (pairs 0..PB-2 are already in oacc)."""
                ws = slice(w * QCH, (w + 1) * QCH)
                bcp = pv_psum.tile([128, QCH], F32, tag="pv", name="bcp")
                nc.tensor.matmul(
                    bcp[64:128, :],
                    ones_row[64:65, :],
                    acc[64:65, ws],
                    start=True,
                    stop=True,
                )
                bcr = io_pool.tile([64, QCH], F32, tag="bcr", bufs=2)
                nc.vector.reciprocal_approx_fast(out=bcr, in_=bcp[64:128, :])
                nc.vector.tensor_mul(
                    out=oacc_tiles[1][64:128, ws], in0=acc[0:64, ws], in1=bcr
                )
                for mt in range(4 * w, 4 * w + 4):
                    po = pv_psum.tile([128, QCH], F32, tag="pv", name="po")
                    for g in range(2):
                        nc.tensor.matmul(
                            po,
                            oacc_tiles[g][:, mt * 128 : (mt + 1) * 128],
                            wot_sb[:, g, :],
                            start=(g == 0),
                            stop=(g == 1),
                            skip_group_check=True,
                        )
                    ot = io_pool.tile([128, HIDDEN], BF16, tag="ot", bufs=4)
                    if mt % 2 == 0:
                        nc.vector.tensor_copy(out=ot, in_=po)
                    else:
                        nc.scalar.copy(out=ot, in_=po)
                    nc.sync.dma_start(
                        out=out_d[mt * 128 : (mt + 1) * 128, :], in_=ot
                    )

            from collections import deque

            tail_ready = []
            pending = deque()
            cur_pv = None
            for idx, job in enumerate(jobs):
                if idx == len(jobs) - 12 and j < PB - 1:
                    prefetch_next_pair(j + 1)
                if job["first"]:
                    cur_pv = pv_psum.tile([128, QCH], F32, tag="pv", name="pv")
                job["pv"] = cur_pv
                if idx > 0 and job["di"] != jobs[idx - 1]["di"]:
                    nxt = None
                    for jb in jobs[idx + 1 :]:
                        if jb["di"] != job["di"]:
                            nxt = jb["di"]
                            break
                    if nxt is not None:
                        get_bt(nxt)
                emit_qk_exp(job)
                pending.append(job)
                if len(pending) > 5:
                    emit_pv(pending.popleft())
            while pending:
                emit_pv(pending.popleft())
            while tail_ready:
                emit_tail_window(tail_ready.pop(0))
            bt_tiles.clear()

            # --- normalization (pairs 0..PB-2): oacc = acc[0:64]*(1/acc[64])
            # 1/w in [32,128] shape via DRAM bounce (a [1,L] DVE op would
            # run on one partition lane); broadcast + multiply on GPSIMD.
            if j < PB - 1:
                nc.sync.dma_start(
                    out=wtmp_d[j].rearrange("p c -> (p c)"), in_=acc[64:65, :]
                )
                wr32 = io_pool.tile([32, 128], F32, tag="wr32", bufs=2)
                nc.sync.dma_start(out=wr32, in_=wtmp_d[j])
                nc.vector.reciprocal_approx_fast(out=wr32, in_=wr32)
                wr32b = io_pool.tile([32, 128], BF16, tag="wr32b", bufs=2)
                nc.vector.tensor_copy(out=wr32b, in_=wr32)
                nc.sync.dma_start(out=wtmp2_d[j], in_=wr32b)
                wrowb = io_pool.tile([1, L], BF16, tag="wrowb", bufs=2)
                nc.sync.dma_start(
                    out=wrowb, in_=wtmp2_d[j].rearrange("p c -> (p c)")
                )
                bc = bc_tiles[j % 2]
                nc.gpsimd.partition_broadcast(bc[:, :], wrowb[0:1, :])
                half = 64 * (j % 2)
                nc.gpsimd.tensor_mul(
                    out=oacc_tiles[j // 2][half : half + 64, :],
                    in0=acc[0:64, :],
                    in1=bc,
                )

    nc.compile()
    return nc


def get_program():
    global _PROGRAM
    if _PROGRAM is None:
        _PROGRAM = build_program()
    return _PROGRAM


def _branch_blob(qT, kT, vv, di):
    """Pack one dilation branch into the [128, W] SBUF-layout blob.

    qT, kT: [64, Ld] transposed Q/K for this branch; vv: [Ld, 65] V plus
    ones column."""
    Ld, nkt = LDS[di], NKTS[di]
    q_part = np.concatenate([qT, qT], axis=0)  # [128, Ld]
    k3 = kT.reshape(64, nkt, 128)
    k_part = np.concatenate(
        [
            k3[:, 0::2, :].reshape(64, -1),
            k3[:, 1::2, :].reshape(64, -1),
        ],
        axis=0,
    )  # [128, Ld/2]
    v_part = vv.reshape(nkt, 128, 128).transpose(1, 0, 2).reshape(128, nkt * 128)
    return np.concatenate([q_part, k_part, v_part], axis=1)


def make_in_maps(query_states, key_states, value_states, Wo):
    q = np.asarray(query_states, dtype=np.float32)
    k = np.asarray(key_states, dtype=np.float32)
    v = np.asarray(value_states, dtype=np.float32)
    Wo = np.asarray(Wo, dtype=np.float32)
    WoT = np.ascontiguousarray(Wo.T)  # [in_hidden, out_hidden]

    in_maps = []
    for c in range(NCORES):
        b, hs = c // 2, (c % 2) * PB
        blob = np.empty((PB, 128, WSUM), BF16_NP)
        for j in range(PB):
            h = hs + j
            for di, d in enumerate(DILS):
                Ld = LDS[di]
                vv = np.zeros((Ld, 128), np.float32)
                vv[:, 0:64] = v[b, h, ::d, :]
                vv[:, 64] = 1.0
                blob[j, :, BOFFS[di] : BOFFS[di] + WS[di]] = _branch_blob(
                    np.ascontiguousarray(q[b, h, ::d, :].T),
                    np.ascontiguousarray(k[b, h, ::d, :].T),
                    vv,
                    di,
                )
        wot = WoT[hs * 64 : (hs + 4) * 64].reshape(2, 128, HIDDEN).astype(BF16_NP)
        in_maps.append({"blob": blob, "wot": wot})
    return in_maps


def combine_outputs(results, bo):
    bo = np.asarray(bo, dtype=np.float32)
    out = np.empty((B, L, HIDDEN), np.float32)
    for b in range(B):
        out[b] = (
            results[2 * b]["out"].astype(np.float32)
            + results[2 * b + 1]["out"].astype(np.float32)
            + bo
        )
    return out


def kernel(
    query_states,
    key_states,
    value_states,
    Wo,
    bo,
    _trace=False,
    _tmpdir=None,
    _results=[None],
):
    from concourse.bass_utils import run_bass_kernel_spmd

    nc = get_program()
    in_maps = make_in_maps(query_states, key_states, value_states, Wo)
    res = run_bass_kernel_spmd(
        nc, in_maps, list(range(NCORES)), trace=_trace, tmpdir=_tmpdir
    )
    _results[0] = res
    return combine_outputs(res.results, bo)
